# revision 19
# baseline (speedup 1.0000x reference)
"""Trainium2 Bass kernel for: ConvTranspose2d(128->256, k=4, s=2, p=1)
-> MaxPool2d(2,2) -> Hardtanh -> spatial mean -> Tanh.

Algebraic restructuring: the stride-2 transposed conv decomposes into 4
polyphase 2x2 convolutions whose outputs at pooled position (i, j) are
exactly the 4 elements of the 2x2 maxpool window, so the 128x128
intermediate is never materialized and everything stays at 64x64.

PE side: fp8(e4m3) matmuls in DoubleRow perf mode.  The pair dimension
carries the two *column* taps of each phase; the two *row* taps are the 2
accumulating matmuls of each PSUM group.  1024 matmuls of N=512.  The
moving canvas is PAIR-INTERLEAVED (two canvases, one per column parity pw;
element layout [pixel][pair], pair stride 1 / column stride 2): measured
140 ns/matmul vs 279 ns for the slab-concatenated layout with strided
pairs.  Keep the a-outer/chunk-inner order — PSUM banks must ROTATE
between consecutive matmuls (back-to-back accumulation pairs on one bank
measured 2.4x slower).  Redundant per-matmul InstLdweights are deduped
post-finalize (LoadStationary overlaps streaming, so this is minor).

Weights are pre-scaled by 32 into fp8 (undone in the final tanh's scale);
phases 2,3 additionally sign-negated (see NEGPH).  PSUM fp32.

Post-processing per 32-row group (4 PSUM banks per phase, pool-rotated
2-deep), in the flipped domain e'' = qb - ps with qb = 32 - 32b, where the
hardtanh clips become 0 <= e'' <= 64 and maxpool max becomes min:
 - phases 0-2: ScalarE act Relu(sgn*ps + qb) folds PSUM read + bias + the
   >=0 clip (sgn per NEGPH);
 - phase 3: DVE dual-op tensor_scalar (ps' + qb) min 64 folds the <=64 cap
   (no reverse-subtract ALU op exists, hence the negated weights);
 - DVE: dvp = max(ev3, 0); c01 = min(ev0, ev1); c23 = min(ev2, dvp) — the
   clamps propagate through the min-tree so u = min(c01, c23) is fully
   clipped with no extra ops;
 - DVE tensor_tensor_reduce fuses the last min with the per-partition
   spatial sum into the group's column of `sums`.
The 3/1 ScalarE/DVE split keeps both engines at ~6 us/group, just above
the PE's ~4.5 us/group, so post-processing is the (mild) bottleneck.
Since sum(clip) rides the flipped domain, the final activation is
tanh(1 - s/(32*4096)) — the correction rides the tanh bias slot.

Sharding: data-parallel over batch, 8 images per core on 8 cores.
"""

import os
import tempfile
from contextlib import ExitStack

import ml_dtypes
import numpy as np

# The neuronx NEFF cache keys on the HLO module hash, which does NOT cover
# the embedded BIR bytes — a process with the same compile history would
# silently reuse a NEFF built from an older version of this kernel.  Point
# the cache at a fresh private dir so every process compiles what it built.
os.environ["NEURON_COMPILE_CACHE_URL"] = tempfile.mkdtemp(prefix="neff-cache-")

import concourse.bacc as bacc
import concourse.bass as bass
import concourse.mybir as mybir
import concourse.tile as tile
from concourse.bass_utils import run_bass_kernel_spmd

# Problem dims (hardcoded per contract)
B, CIN, COUT, H, W = 64, 128, 256, 64, 64
NCORES = 8
BPC = B // NCORES  # images per core

NROW = 66          # padded rows (1 + 64 + 1)
WSLAB = NROW * 64  # 4224 elements per dj-slab
NSLAB = 3          # dj in {-1, 0, +1}
CVTOT = 2 * 2 * WSLAB  # two pair-interleaved canvases (one per column parity)

WSCALE = 32.0      # fp8 weight pre-scale; undone in the final tanh

NCHK = int(os.environ.get("KNCHK", "4"))  # 8-row chunks per group (1 PSUM bank each)
NGRP = 8 // NCHK   # groups of NCHK*8 output rows per (img, half)

# Phases whose weights are sign-negated on the host: phase 2 so its ScalarE
# evacuation uses scale=+1 (cosmetic), phase 3 so the DVE can evacuate it as
# (ps' + qb) min 64 with a dual-op tensor_scalar (there is no reverse-subtract
# ALU op, so qb - ps needs negated weights).
NEGPH = (2, 3)

F32 = mybir.dt.float32
BF16 = mybir.dt.bfloat16
FP8 = mybir.dt.float8e4


def _tap(ph: int, a: int):
    """For phase parity ph (0=even output coord, 1=odd) and tap index a,
    return (input shift, kernel index) in one dimension.

    ConvTranspose2d(stride=2, pad=1): out[2q+r] = sum over taps of
    x[q+di] * w[k].  r=0: (di,k) in {(0,1), (-1,3)}; r=1: {(1,0), (0,2)}.
    """
    if ph == 0:
        return (0, 1) if a == 0 else (-1, 3)
    return (1, 0) if a == 0 else (0, 2)


def _kw_pair(pw: int):
    """kw for DoubleRow pair slots (i=0, i=1); i indexes adjacent dj-slabs
    starting at _slab_lo(pw), i.e. i=0 is the smaller dj."""
    return (3, 1) if pw == 0 else (2, 0)


def _slab_lo(pw: int) -> int:
    """First dj-slab of the pair for column parity pw (slab s = dj+1)."""
    return 0 if pw == 0 else 1


def build_nc(
    n_imgs: int = BPC,
    repeat: int = 1,
    pe_only: bool | None = None,
    mmfree: int | None = None,
) -> bass.Bass:
    """repeat>1 wraps the whole compute in a hardware loop executing it
    `repeat` times — used only for wall-clock timing (amortizes the ~80ms
    axon RPC overhead); the graded path uses repeat=1 (no loop).

    pe_only drops all post-processing (timing experiment; wrong results).
    mmfree is the output free-dim size per matmul (512 is the ISA max;
    larger values fail walrus codegen's is_valid_s3d3_mm check).
    post: 'old' = all 4 phase evacuations on ScalarE;
          'split' = evacuations alternate ScalarE/DVE and the pool-min
          tree moves partly to GpSimd, so no single engine gates the PE."""
    if pe_only is None:
        pe_only = os.environ.get("KPE_ONLY", "0") == "1"
    if mmfree is None:
        mmfree = int(os.environ.get("KMMFREE", "512"))
    assert (NCHK * 512) % mmfree == 0
    post = os.environ.get("KPOST", "split")
    nc = bacc.Bacc("TRN2", target_bir_lowering=False, debug=False)

    xc = nc.dram_tensor("xc", [BPC, 128, CVTOT], FP8, kind="ExternalInput")
    wm = nc.dram_tensor("wm", [128, 16 * 256], FP8, kind="ExternalInput")
    qbr = nc.dram_tensor("qb", [128, 2], F32, kind="ExternalInput")
    out = nc.dram_tensor("out", [128, 2 * BPC], F32, kind="ExternalOutput")

    Relu = mybir.ActivationFunctionType.Relu
    Tanh = mybir.ActivationFunctionType.Tanh
    MIN = mybir.AluOpType.min
    ADD = mybir.AluOpType.add
    DR = mybir.MatmulPerfMode.DoubleRow

    with ExitStack() as ctx:
        tc = ctx.enter_context(tile.TileContext(nc))
        consts = ctx.enter_context(tc.tile_pool(name="consts", bufs=1))
        canvp = ctx.enter_context(tc.tile_pool(name="canv", bufs=3))
        psump = ctx.enter_context(
            tc.tile_pool(name="ps", bufs=8 // NCHK, space="PSUM")
        )
        evacp = ctx.enter_context(tc.tile_pool(name="ev", bufs=8))
        mpool = ctx.enter_context(tc.tile_pool(name="mt", bufs=2))

        w_sb = consts.tile([128, 16 * 256], FP8, tag="w")
        nc.sync.dma_start(w_sb[:], wm[:, :])
        qb_sb = consts.tile([128, 2], F32, tag="qb")
        nc.sync.dma_start(qb_sb[:], qbr[:, :])
        sums = consts.tile([128, 2 * BPC * NGRP], F32, tag="sums")
        nc.vector.memset(sums[:], 0.0)
        s_red = consts.tile([128, 2 * BPC], F32, tag="sred")
        o_sb = consts.tile([128, 2 * BPC], F32, tag="out")

        nf = NCHK * 512

        def body():
            for img in range(n_imgs):
                canv = canvp.tile([128, CVTOT], FP8, tag="canv")
                nc.sync.dma_start(canv[:], xc[img])
                # two pair-interleaved canvases, one per column parity pw:
                # cvi[pw][p, i, x] = slab[lo(pw)+i][p, x], pair stride 1,
                # column stride 2 (the layout the PE streams fastest).
                cvi = [
                    canv[:, pw * 2 * WSLAB : (pw + 1) * 2 * WSLAB].rearrange(
                        "p (x i) -> p i x", i=2
                    )
                    for pw in range(2)
                ]
                for half in range(2):
                    qb_ap = qb_sb[:, half : half + 1]
                    for g in range(NGRP):
                        evs = []
                        for phase in range(4):
                            ph, pw = phase >> 1, phase & 1
                            ps = psump.tile([128, NCHK, 512], F32, tag="ps")
                            psf = ps[:].rearrange("p a b -> p (a b)")
                            for a in range(2):
                                di, _kh = _tap(ph, a)
                                s = (half * 4 + phase) * 2 + a
                                w_ap = w_sb[
                                    :, s * 256 : (s + 1) * 256
                                ].rearrange("p (i m) -> p i m", i=2)
                                for c in range(NCHK * 512 // mmfree):
                                    r0 = (mmfree // 64) * (
                                        (NCHK * 512 // mmfree) * g + c
                                    )
                                    off = (1 + r0 + di) * 64
                                    noacc = pe_only and (
                                        os.environ.get("KNOACC", "0") == "1"
                                    )
                                    nc.tensor.matmul(
                                        psf[:, c * mmfree : (c + 1) * mmfree],
                                        w_ap,
                                        cvi[pw][:, :, off : off + mmfree],
                                        start=(a == 0) or noacc,
                                        stop=(a == 1) or noacc,
                                        perf_mode=DR,
                                        skip_group_check=True,
                                    )
                            if pe_only:
                                # timing experiment: a 1-element read is the
                                # cheapest consumer that still releases the
                                # PSUM tile for pool reuse.
                                nc.vector.tensor_scalar(
                                    sums[:, :1], ps[:, 0, :1], 0.0, None, ADD
                                )
                                continue
                            # Evacuate each phase in the flipped domain
                            # e'' = qb - ps with qb = 32 - 32b: the upper
                            # hardtanh clip is e'' >= 0, the lower is
                            # e'' <= 64.  Phases 0-2 go through ScalarE as
                            # Relu(sgn*ps + qb) (folds the >=0 clip); phase 3
                            # (weights negated) goes through the DVE as
                            # (ps' + qb) min 64 — a dual-op tensor_scalar
                            # with per-partition scalar — folding the <=64
                            # cap instead.  Splitting 3/1 keeps both engines
                            # under the PE's per-group time.
                            sgn = 1.0 if phase in NEGPH else -1.0
                            ev = evacp.tile([128, nf], BF16, tag="ev")
                            if post == "split" and phase == 3:
                                nc.vector.tensor_scalar(
                                    ev[:], psf, qb_ap, 64.0, ADD, MIN,
                                )
                            else:
                                nc.scalar.activation(
                                    ev[:], psf, Relu, bias=qb_ap, scale=sgn,
                                )
                            evs.append(ev)

                        if pe_only:
                            continue
                        col = (img * 2 + half) * NGRP + g
                        if post == "split":
                            # maxpool max == min over e''.  ev3 is <=64 but
                            # unclamped below; ev0-2 are >=0 but uncapped.
                            # dvp = max(ev3, 0) completes ev3's clamp, so the
                            # min-tree result u = min(c01, c23) lands in
                            # [0, 64] with no further clamps, and the final
                            # tensor_tensor_reduce fuses u's min with the
                            # per-partition spatial sum.
                            dvp = mpool.tile([128, nf], BF16, tag="dvp")
                            nc.vector.tensor_scalar(
                                dvp[:], evs[3][:], 0.0, None,
                                mybir.AluOpType.max,
                            )
                            c1 = mpool.tile([128, nf], BF16, tag="c1")
                            nc.vector.tensor_tensor(
                                c1[:], evs[0][:], evs[1][:], MIN
                            )
                            c2 = mpool.tile([128, nf], BF16, tag="c2")
                            nc.vector.tensor_tensor(
                                c2[:], evs[2][:], dvp[:], MIN
                            )
                            # NOTE: tensor_tensor_reduce would fuse the last
                            # min with the accumulating sum, but it hangs the
                            # device (mesh desync) despite passing CoreSim —
                            # keep it opt-in for future investigation.
                            u = mpool.tile([128, nf], BF16, tag="u")
                            if os.environ.get("KTTR", "0") == "1":
                                nc.vector.tensor_tensor_reduce(
                                    u[:], c1[:], c2[:], 1.0, 0.0, MIN, ADD,
                                    accum_out=sums[:, col : col + 1],
                                )
                            else:
                                nc.vector.tensor_tensor(
                                    u[:], c1[:], c2[:], MIN
                                )
                                wcl = mpool.tile([128, nf], BF16, tag="wcl")
                                nc.vector.tensor_scalar(
                                    wcl[:], u[:], 64.0, None, MIN, ADD,
                                    accum_out=sums[:, col : col + 1],
                                )
                        else:
                            c1 = mpool.tile([128, nf], BF16, tag="c1")
                            nc.vector.tensor_tensor(
                                c1[:], evs[0][:], evs[1][:], MIN
                            )
                            c2 = mpool.tile([128, nf], BF16, tag="c2")
                            nc.vector.tensor_tensor(
                                c2[:], evs[2][:], evs[3][:], MIN
                            )
                            u = mpool.tile([128, nf], BF16, tag="u")
                            nc.vector.tensor_tensor(u[:], c1[:], c2[:], MIN)
                            wcl = mpool.tile([128, nf], BF16, tag="wcl")
                            nc.vector.tensor_scalar(
                                wcl[:],
                                u[:],
                                64.0,
                                None,
                                MIN,
                                ADD,
                                accum_out=sums[:, col : col + 1],
                            )

        if repeat > 1:
            with tc.For_i(0, repeat, 1):
                body()
        else:
            body()

        nc.vector.tensor_reduce(
            s_red[:],
            sums[:].rearrange("p (i g) -> p i g", g=NGRP),
            axis=mybir.AxisListType.X,
            op=ADD,
        )
        # sum(w) = 32*4096 - sum(min-tree)  =>  mean/32 = 1 - s/(32*4096)
        nc.scalar.activation(
            o_sb[:], s_red[:], Tanh, scale=-1.0 / (WSCALE * 4096.0), bias=1.0
        )
        nc.sync.dma_start(out[:, :], o_sb[:])

    nc.finalize()
    if os.environ.get("KDEDUP", "1") == "1":
        _dedup_ldweights(nc)
    return nc


def _dedup_ldweights(nc: bass.Bass) -> int:
    """Remove redundant InstLdweights from the finalized module.

    Bass legalization emits one InstLdweights per InstMatmult even when
    consecutive matmuls use identical stationary weights; each load costs
    ~256 PE cycles serialized on the PE queue.  Consecutive matmuls in this
    kernel reuse the same weights 4x (the NCHK chunk loop), so 3/4 of the
    loads are dead.  Only wait/update-free duplicates are removed; tracking
    resets at block boundaries so loop bodies stay self-contained.
    """
    removed = 0
    for blk in nc.m.functions[0].blocks:
        last = None
        keep = []
        for inst in blk.instructions:
            tn = type(inst).__name__
            if tn == "InstLdweights":
                sig = (
                    str(inst.ins[0]),
                    str(inst.perf_mode),
                    str(inst.is_transpose),
                    str(inst.tile_position),
                    str(inst.tile_size),
                )
                si = inst.sync_info
                clean = si is None or (
                    len(si.on_wait) == 0 and len(si.on_update) == 0
                )
                if sig == last and clean:
                    removed += 1
                    continue
                last = sig
            keep.append(inst)
        blk.instructions = keep
    return removed


_CACHE: dict = {}


def _get_nc() -> bass.Bass:
    if "nc" not in _CACHE:
        _CACHE["nc"] = build_nc()
    return _CACHE["nc"]


def make_in_maps(x: np.ndarray, weight: np.ndarray, bias: np.ndarray):
    x = np.asarray(x, dtype=np.float32)
    weight = np.asarray(weight, dtype=np.float32)
    bias = np.asarray(bias, dtype=np.float32)
    f8 = ml_dtypes.float8_e4m3

    xq = x.astype(f8)  # |x| << 240, no clipping needed
    # 3 column-shifted zero-padded copies: canv[b,s,p,1+r,c] = x[b,p,r,c+dj],
    # slab s = dj+1.
    canv = np.zeros((B, NSLAB, 128, NROW, 64), dtype=f8)
    canv[:, 1, :, 1:65, :] = xq
    canv[:, 0, :, 1:65, 1:64] = xq[:, :, :, 0:63]
    canv[:, 2, :, 1:65, 0:63] = xq[:, :, :, 1:64]
    # Pair-interleave per column parity pw: cvi[b,p,pw,x,i] = slab[lo(pw)+i]
    # at pixel x.  Pair stride 1 / column stride 2 is the moving layout the
    # PE streams fastest (measured 140 ns vs 279 ns per matmul for the
    # slab-concatenated layout).
    sl = canv.reshape(B, NSLAB, 128, WSLAB)
    cvi = np.zeros((B, 128, 2, WSLAB, 2), dtype=f8)
    for pw in range(2):
        lo = _slab_lo(pw)
        cvi[:, :, pw, :, 0] = sl[:, lo]
        cvi[:, :, pw, :, 1] = sl[:, lo + 1]
    canvf = np.ascontiguousarray(cvi).reshape(B, 128, CVTOT)

    wq = np.clip(weight * WSCALE, -240.0, 240.0).astype(f8)  # [cin,cout,kh,kw]
    wmv = np.zeros((128, 16 * 256), dtype=f8)
    for half in range(2):
        blk = wq[:, half * 128 : (half + 1) * 128]  # [128,128,4,4]
        for phase in range(4):
            ph, pw = phase >> 1, phase & 1
            kw0, kw1 = _kw_pair(pw)
            # NEGPH phases are sign-negated so their PSUM can be evacuated
            # as Relu(+ps + qb) (see build_nc).
            psgn = -1.0 if phase in NEGPH else 1.0
            for a in range(2):
                _di, kh = _tap(ph, a)
                s = (half * 4 + phase) * 2 + a
                wmv[:, s * 256 : s * 256 + 128] = psgn * blk[:, :, kh, kw0]
                wmv[:, s * 256 + 128 : s * 256 + 256] = psgn * blk[:, :, kh, kw1]

    qbv = np.ascontiguousarray(
        (WSCALE - WSCALE * bias.reshape(2, 128).T), dtype=np.float32
    )

    return [
        {"xc": canvf[c * BPC : (c + 1) * BPC], "wm": wmv, "qb": qbv}
        for c in range(NCORES)
    ]


def assemble_output(results: list) -> np.ndarray:
    outs = []
    for c in range(NCORES):
        o = np.asarray(results[c]["out"])  # [128, 2*BPC]
        o = o.reshape(128, BPC, 2).transpose(1, 2, 0).reshape(BPC, COUT)
        outs.append(o)
    return np.concatenate(outs, 0).reshape(B, COUT, 1, 1).astype(np.float32)


def kernel(x: np.ndarray, weight: np.ndarray, bias: np.ndarray) -> np.ndarray:
    nc = _get_nc()
    in_maps = make_in_maps(x, weight, bias)
    res = run_bass_kernel_spmd(nc, in_maps, core_ids=list(range(NCORES)))
    return assemble_output(res.results)



# revision 21
# speedup vs baseline: 1.1394x; 1.1394x over previous
"""Trainium2 Bass kernel for: ConvTranspose2d(128->256, k=4, s=2, p=1)
-> MaxPool2d(2,2) -> Hardtanh -> spatial mean -> Tanh.

Algebraic restructuring: the stride-2 transposed conv decomposes into 4
polyphase 2x2 convolutions whose outputs at pooled position (i, j) are
exactly the 4 elements of the 2x2 maxpool window, so the 128x128
intermediate is never materialized and everything stays at 64x64.

PE side: fp8(e4m3) matmuls in DoubleRow perf mode.  The pair dimension
carries the two *column* taps of each phase; the two *row* taps are the 2
accumulating matmuls of each PSUM group.  1024 matmuls of N=512.  The
moving canvas is PAIR-INTERLEAVED (two canvases, one per column parity pw;
element layout [pixel][pair], pair stride 1 / column stride 2): measured
140 ns/matmul vs 279 ns for the slab-concatenated layout with strided
pairs.  Keep the a-outer/chunk-inner order — PSUM banks must ROTATE
between consecutive matmuls (back-to-back accumulation pairs on one bank
measured 2.4x slower).  Redundant per-matmul InstLdweights are deduped
post-finalize (LoadStationary overlaps streaming, so this is minor).

Weights are pre-scaled by 32 into fp8 (undone in the final tanh's scale);
phases 2,3 additionally sign-negated (see NEGPH).  PSUM fp32.

Post-processing per 32-row group (4 PSUM banks per phase, pool-rotated
2-deep), in the flipped domain e'' = qb - ps with qb = 32 - 32b, where the
hardtanh clips become 0 <= e'' <= 64 and maxpool max becomes min.
Default mode 'old': ScalarE evacuates every phase as Relu(sgn*ps + qb)
(folds PSUM read + bias + the >=0 clip; sgn per NEGPH) and the DVE runs
the 3-op min-tree plus a final (min 64)+accumulate tensor_scalar that
yields the spatial sum.  Keeping ALL PSUM evacuations on ScalarE matters:
engine queues are in-order, so an evacuation placed on the DVE gets stuck
behind the previous group's min-tree and stalls the PE on PSUM reuse
(the 'split' mode that shares evacuations 3/1 measured 13% slower).
Since sum(clip) rides the flipped domain, the final activation is
tanh(1 - s/(32*4096)) — the correction rides the tanh bias slot.

Sharding: data-parallel over batch, 8 images per core on 8 cores.
"""

import os
import tempfile
from contextlib import ExitStack

import ml_dtypes
import numpy as np

# The neuronx NEFF cache keys on the HLO module hash, which does NOT cover
# the embedded BIR bytes — a process with the same compile history would
# silently reuse a NEFF built from an older version of this kernel.  Point
# the cache at a fresh private dir so every process compiles what it built.
os.environ["NEURON_COMPILE_CACHE_URL"] = tempfile.mkdtemp(prefix="neff-cache-")

import concourse.bacc as bacc
import concourse.bass as bass
import concourse.mybir as mybir
import concourse.tile as tile
from concourse.bass_utils import run_bass_kernel_spmd

# Problem dims (hardcoded per contract)
B, CIN, COUT, H, W = 64, 128, 256, 64, 64
NCORES = 8
BPC = B // NCORES  # images per core

NROW = 66          # padded rows (1 + 64 + 1)
WSLAB = NROW * 64  # 4224 elements per dj-slab
NSLAB = 3          # dj in {-1, 0, +1}
CVTOT = 2 * 2 * WSLAB  # two pair-interleaved canvases (one per column parity)

WSCALE = 32.0      # fp8 weight pre-scale; undone in the final tanh

NCHK = int(os.environ.get("KNCHK", "4"))  # 8-row chunks per group (1 PSUM bank each)
NGRP = 8 // NCHK   # groups of NCHK*8 output rows per (img, half)

# Phases whose weights are sign-negated on the host: phase 2 so its ScalarE
# evacuation uses scale=+1 (cosmetic), phase 3 so the DVE can evacuate it as
# (ps' + qb) min 64 with a dual-op tensor_scalar (there is no reverse-subtract
# ALU op, so qb - ps needs negated weights).
NEGPH = (2, 3)

F32 = mybir.dt.float32
BF16 = mybir.dt.bfloat16
FP8 = mybir.dt.float8e4


def _tap(ph: int, a: int):
    """For phase parity ph (0=even output coord, 1=odd) and tap index a,
    return (input shift, kernel index) in one dimension.

    ConvTranspose2d(stride=2, pad=1): out[2q+r] = sum over taps of
    x[q+di] * w[k].  r=0: (di,k) in {(0,1), (-1,3)}; r=1: {(1,0), (0,2)}.
    """
    if ph == 0:
        return (0, 1) if a == 0 else (-1, 3)
    return (1, 0) if a == 0 else (0, 2)


def _kw_pair(pw: int):
    """kw for DoubleRow pair slots (i=0, i=1); i indexes adjacent dj-slabs
    starting at _slab_lo(pw), i.e. i=0 is the smaller dj."""
    return (3, 1) if pw == 0 else (2, 0)


def _slab_lo(pw: int) -> int:
    """First dj-slab of the pair for column parity pw (slab s = dj+1)."""
    return 0 if pw == 0 else 1


def build_nc(
    n_imgs: int = BPC,
    repeat: int = 1,
    pe_only: bool | None = None,
    mmfree: int | None = None,
) -> bass.Bass:
    """repeat>1 wraps the whole compute in a hardware loop executing it
    `repeat` times — used only for wall-clock timing (amortizes the ~80ms
    axon RPC overhead); the graded path uses repeat=1 (no loop).

    pe_only drops all post-processing (timing experiment; wrong results).
    mmfree is the output free-dim size per matmul (512 is the ISA max;
    larger values fail walrus codegen's is_valid_s3d3_mm check).
    post: 'old' = all 4 phase evacuations on ScalarE;
          'split' = evacuations alternate ScalarE/DVE and the pool-min
          tree moves partly to GpSimd, so no single engine gates the PE."""
    if pe_only is None:
        pe_only = os.environ.get("KPE_ONLY", "0") == "1"
    if mmfree is None:
        mmfree = int(os.environ.get("KMMFREE", "512"))
    assert (NCHK * 512) % mmfree == 0
    # 'old' (all evacuations on ScalarE) measured faster than 'split' in a
    # same-process interleaved A/B (305 vs 346 us): the DVE's in-order queue
    # delays split's PSUM-releasing evacuation behind the previous group's
    # min-tree, stalling the PE.  ScalarE has no such coupling.
    post = os.environ.get("KPOST", "old")
    nc = bacc.Bacc("TRN2", target_bir_lowering=False, debug=False)

    xc = nc.dram_tensor("xc", [BPC, 128, CVTOT], FP8, kind="ExternalInput")
    wm = nc.dram_tensor("wm", [128, 16 * 256], FP8, kind="ExternalInput")
    qbr = nc.dram_tensor("qb", [128, 2], F32, kind="ExternalInput")
    out = nc.dram_tensor("out", [128, 2 * BPC], F32, kind="ExternalOutput")

    Relu = mybir.ActivationFunctionType.Relu
    Tanh = mybir.ActivationFunctionType.Tanh
    MIN = mybir.AluOpType.min
    ADD = mybir.AluOpType.add
    DR = mybir.MatmulPerfMode.DoubleRow

    with ExitStack() as ctx:
        tc = ctx.enter_context(tile.TileContext(nc))
        consts = ctx.enter_context(tc.tile_pool(name="consts", bufs=1))
        canvp = ctx.enter_context(tc.tile_pool(name="canv", bufs=3))
        psump = ctx.enter_context(
            tc.tile_pool(name="ps", bufs=8 // NCHK, space="PSUM")
        )
        evacp = ctx.enter_context(tc.tile_pool(name="ev", bufs=8))
        mpool = ctx.enter_context(tc.tile_pool(name="mt", bufs=2))

        w_sb = consts.tile([128, 16 * 256], FP8, tag="w")
        nc.sync.dma_start(w_sb[:], wm[:, :])
        qb_sb = consts.tile([128, 2], F32, tag="qb")
        nc.sync.dma_start(qb_sb[:], qbr[:, :])
        sums = consts.tile([128, 2 * BPC * NGRP], F32, tag="sums")
        nc.vector.memset(sums[:], 0.0)
        s_red = consts.tile([128, 2 * BPC], F32, tag="sred")
        o_sb = consts.tile([128, 2 * BPC], F32, tag="out")

        nf = NCHK * 512

        def body():
            for img in range(n_imgs):
                canv = canvp.tile([128, CVTOT], FP8, tag="canv")
                nc.sync.dma_start(canv[:], xc[img])
                # two pair-interleaved canvases, one per column parity pw:
                # cvi[pw][p, i, x] = slab[lo(pw)+i][p, x], pair stride 1,
                # column stride 2 (the layout the PE streams fastest).
                cvi = [
                    canv[:, pw * 2 * WSLAB : (pw + 1) * 2 * WSLAB].rearrange(
                        "p (x i) -> p i x", i=2
                    )
                    for pw in range(2)
                ]
                for half in range(2):
                    qb_ap = qb_sb[:, half : half + 1]
                    for g in range(NGRP):
                        evs = []
                        for phase in range(4):
                            ph, pw = phase >> 1, phase & 1
                            ps = psump.tile([128, NCHK, 512], F32, tag="ps")
                            psf = ps[:].rearrange("p a b -> p (a b)")
                            for a in range(2):
                                di, _kh = _tap(ph, a)
                                s = (half * 4 + phase) * 2 + a
                                w_ap = w_sb[
                                    :, s * 256 : (s + 1) * 256
                                ].rearrange("p (i m) -> p i m", i=2)
                                for c in range(NCHK * 512 // mmfree):
                                    r0 = (mmfree // 64) * (
                                        (NCHK * 512 // mmfree) * g + c
                                    )
                                    off = (1 + r0 + di) * 64
                                    noacc = pe_only and (
                                        os.environ.get("KNOACC", "0") == "1"
                                    )
                                    nc.tensor.matmul(
                                        psf[:, c * mmfree : (c + 1) * mmfree],
                                        w_ap,
                                        cvi[pw][:, :, off : off + mmfree],
                                        start=(a == 0) or noacc,
                                        stop=(a == 1) or noacc,
                                        perf_mode=DR,
                                        skip_group_check=True,
                                    )
                            if pe_only:
                                # timing experiment: a 1-element read is the
                                # cheapest consumer that still releases the
                                # PSUM tile for pool reuse.
                                nc.vector.tensor_scalar(
                                    sums[:, :1], ps[:, 0, :1], 0.0, None, ADD
                                )
                                continue
                            # Evacuate each phase in the flipped domain
                            # e'' = qb - ps with qb = 32 - 32b: the upper
                            # hardtanh clip is e'' >= 0, the lower is
                            # e'' <= 64.  Phases 0-2 go through ScalarE as
                            # Relu(sgn*ps + qb) (folds the >=0 clip); phase 3
                            # (weights negated) goes through the DVE as
                            # (ps' + qb) min 64 — a dual-op tensor_scalar
                            # with per-partition scalar — folding the <=64
                            # cap instead.  Splitting 3/1 keeps both engines
                            # under the PE's per-group time.
                            sgn = 1.0 if phase in NEGPH else -1.0
                            ev = evacp.tile([128, nf], BF16, tag="ev")
                            if post == "split" and phase == 3:
                                nc.vector.tensor_scalar(
                                    ev[:], psf, qb_ap, 64.0, ADD, MIN,
                                )
                            else:
                                nc.scalar.activation(
                                    ev[:], psf, Relu, bias=qb_ap, scale=sgn,
                                )
                            evs.append(ev)

                        if pe_only:
                            continue
                        col = (img * 2 + half) * NGRP + g
                        if post == "split":
                            # maxpool max == min over e''.  ev3 is <=64 but
                            # unclamped below; ev0-2 are >=0 but uncapped.
                            # dvp = max(ev3, 0) completes ev3's clamp, so the
                            # min-tree result u = min(c01, c23) lands in
                            # [0, 64] with no further clamps, and the final
                            # tensor_tensor_reduce fuses u's min with the
                            # per-partition spatial sum.
                            dvp = mpool.tile([128, nf], BF16, tag="dvp")
                            nc.vector.tensor_scalar(
                                dvp[:], evs[3][:], 0.0, None,
                                mybir.AluOpType.max,
                            )
                            c1 = mpool.tile([128, nf], BF16, tag="c1")
                            nc.vector.tensor_tensor(
                                c1[:], evs[0][:], evs[1][:], MIN
                            )
                            c2 = mpool.tile([128, nf], BF16, tag="c2")
                            nc.vector.tensor_tensor(
                                c2[:], evs[2][:], dvp[:], MIN
                            )
                            # NOTE: tensor_tensor_reduce would fuse the last
                            # min with the accumulating sum, but it hangs the
                            # device (mesh desync) despite passing CoreSim —
                            # keep it opt-in for future investigation.
                            u = mpool.tile([128, nf], BF16, tag="u")
                            if os.environ.get("KTTR", "0") == "1":
                                nc.vector.tensor_tensor_reduce(
                                    u[:], c1[:], c2[:], 1.0, 0.0, MIN, ADD,
                                    accum_out=sums[:, col : col + 1],
                                )
                            else:
                                nc.vector.tensor_tensor(
                                    u[:], c1[:], c2[:], MIN
                                )
                                wcl = mpool.tile([128, nf], BF16, tag="wcl")
                                nc.vector.tensor_scalar(
                                    wcl[:], u[:], 64.0, None, MIN, ADD,
                                    accum_out=sums[:, col : col + 1],
                                )
                        else:
                            c1 = mpool.tile([128, nf], BF16, tag="c1")
                            nc.vector.tensor_tensor(
                                c1[:], evs[0][:], evs[1][:], MIN
                            )
                            c2 = mpool.tile([128, nf], BF16, tag="c2")
                            nc.vector.tensor_tensor(
                                c2[:], evs[2][:], evs[3][:], MIN
                            )
                            u = mpool.tile([128, nf], BF16, tag="u")
                            nc.vector.tensor_tensor(u[:], c1[:], c2[:], MIN)
                            wcl = mpool.tile([128, nf], BF16, tag="wcl")
                            nc.vector.tensor_scalar(
                                wcl[:],
                                u[:],
                                64.0,
                                None,
                                MIN,
                                ADD,
                                accum_out=sums[:, col : col + 1],
                            )

        if repeat > 1:
            with tc.For_i(0, repeat, 1):
                body()
        else:
            body()

        nc.vector.tensor_reduce(
            s_red[:],
            sums[:].rearrange("p (i g) -> p i g", g=NGRP),
            axis=mybir.AxisListType.X,
            op=ADD,
        )
        # sum(w) = 32*4096 - sum(min-tree)  =>  mean/32 = 1 - s/(32*4096)
        nc.scalar.activation(
            o_sb[:], s_red[:], Tanh, scale=-1.0 / (WSCALE * 4096.0), bias=1.0
        )
        nc.sync.dma_start(out[:, :], o_sb[:])

    nc.finalize()
    if os.environ.get("KDEDUP", "1") == "1":
        _dedup_ldweights(nc)
    return nc


def _dedup_ldweights(nc: bass.Bass) -> int:
    """Remove redundant InstLdweights from the finalized module.

    Bass legalization emits one InstLdweights per InstMatmult even when
    consecutive matmuls use identical stationary weights; each load costs
    ~256 PE cycles serialized on the PE queue.  Consecutive matmuls in this
    kernel reuse the same weights 4x (the NCHK chunk loop), so 3/4 of the
    loads are dead.  Only wait/update-free duplicates are removed; tracking
    resets at block boundaries so loop bodies stay self-contained.
    """
    removed = 0
    for blk in nc.m.functions[0].blocks:
        last = None
        keep = []
        for inst in blk.instructions:
            tn = type(inst).__name__
            if tn == "InstLdweights":
                sig = (
                    str(inst.ins[0]),
                    str(inst.perf_mode),
                    str(inst.is_transpose),
                    str(inst.tile_position),
                    str(inst.tile_size),
                )
                si = inst.sync_info
                clean = si is None or (
                    len(si.on_wait) == 0 and len(si.on_update) == 0
                )
                if sig == last and clean:
                    removed += 1
                    continue
                last = sig
            keep.append(inst)
        blk.instructions = keep
    return removed


_CACHE: dict = {}


def _get_nc() -> bass.Bass:
    if "nc" not in _CACHE:
        _CACHE["nc"] = build_nc()
    return _CACHE["nc"]


def make_in_maps(x: np.ndarray, weight: np.ndarray, bias: np.ndarray):
    x = np.asarray(x, dtype=np.float32)
    weight = np.asarray(weight, dtype=np.float32)
    bias = np.asarray(bias, dtype=np.float32)
    f8 = ml_dtypes.float8_e4m3

    xq = x.astype(f8)  # |x| << 240, no clipping needed
    # 3 column-shifted zero-padded copies: canv[b,s,p,1+r,c] = x[b,p,r,c+dj],
    # slab s = dj+1.
    canv = np.zeros((B, NSLAB, 128, NROW, 64), dtype=f8)
    canv[:, 1, :, 1:65, :] = xq
    canv[:, 0, :, 1:65, 1:64] = xq[:, :, :, 0:63]
    canv[:, 2, :, 1:65, 0:63] = xq[:, :, :, 1:64]
    # Pair-interleave per column parity pw: cvi[b,p,pw,x,i] = slab[lo(pw)+i]
    # at pixel x.  Pair stride 1 / column stride 2 is the moving layout the
    # PE streams fastest (measured 140 ns vs 279 ns per matmul for the
    # slab-concatenated layout).
    sl = canv.reshape(B, NSLAB, 128, WSLAB)
    cvi = np.zeros((B, 128, 2, WSLAB, 2), dtype=f8)
    for pw in range(2):
        lo = _slab_lo(pw)
        cvi[:, :, pw, :, 0] = sl[:, lo]
        cvi[:, :, pw, :, 1] = sl[:, lo + 1]
    canvf = np.ascontiguousarray(cvi).reshape(B, 128, CVTOT)

    wq = np.clip(weight * WSCALE, -240.0, 240.0).astype(f8)  # [cin,cout,kh,kw]
    wmv = np.zeros((128, 16 * 256), dtype=f8)
    for half in range(2):
        blk = wq[:, half * 128 : (half + 1) * 128]  # [128,128,4,4]
        for phase in range(4):
            ph, pw = phase >> 1, phase & 1
            kw0, kw1 = _kw_pair(pw)
            # NEGPH phases are sign-negated so their PSUM can be evacuated
            # as Relu(+ps + qb) (see build_nc).
            psgn = -1.0 if phase in NEGPH else 1.0
            for a in range(2):
                _di, kh = _tap(ph, a)
                s = (half * 4 + phase) * 2 + a
                wmv[:, s * 256 : s * 256 + 128] = psgn * blk[:, :, kh, kw0]
                wmv[:, s * 256 + 128 : s * 256 + 256] = psgn * blk[:, :, kh, kw1]

    qbv = np.ascontiguousarray(
        (WSCALE - WSCALE * bias.reshape(2, 128).T), dtype=np.float32
    )

    return [
        {"xc": canvf[c * BPC : (c + 1) * BPC], "wm": wmv, "qb": qbv}
        for c in range(NCORES)
    ]


def assemble_output(results: list) -> np.ndarray:
    outs = []
    for c in range(NCORES):
        o = np.asarray(results[c]["out"])  # [128, 2*BPC]
        o = o.reshape(128, BPC, 2).transpose(1, 2, 0).reshape(BPC, COUT)
        outs.append(o)
    return np.concatenate(outs, 0).reshape(B, COUT, 1, 1).astype(np.float32)


def kernel(x: np.ndarray, weight: np.ndarray, bias: np.ndarray) -> np.ndarray:
    nc = _get_nc()
    in_maps = make_in_maps(x, weight, bias)
    res = run_bass_kernel_spmd(nc, in_maps, core_ids=list(range(NCORES)))
    return assemble_output(res.results)

